# revision 18
# baseline (speedup 1.0000x reference)
"""Trainium2 Bass kernel for nn_Attention (B=4, N=2048, D=1024, H=16, Dh=64).

Distribution over 8 NeuronCores: 4-way data parallel on batch x 2-way tensor
parallel on heads (8 heads / 512 inner dims per core). Each core computes a
partial output projection; the host sums the two head-group partials per batch
and adds bo.

Per-core dataflow (all matmuls f32r or bf16 at 1 cycle/row on the PE):
  A) x-LN (mean/var only; ln_w/ln_b folded into the projection weights on the
     host) -> PE-transpose x_ln -> XT [D, tokens]
  A2) Q/K/V token-major via XT-stationary matmuls. qk-LN mean is removed by
     host-side per-head column centering of Wq/Wk, so only sumsq/rstd is
     computed on device. Q/K are then PE-transposed into QT/KT [inner, tokens].
  B) attention in S^T layout: S^T[keys,q] = K @ Q^T, exp on ACT (no max
     subtraction -- |score| <= 8 by LN bounds), causal mask as 0/1 multiply on
     diagonal tiles, P^T@V with a fused ones-column for the softmax denominator,
     normalization via gpsimd partition-broadcast of 1/denom.
  C) out-proj: out = (O^T)^T @ Wo in bf16.
"""

import os
import sys
import types

import numpy as np
import ml_dtypes

B, N, D = 4, 2048, 1024
H, Dh = 16, 64
HL, IL = 8, 512          # local heads / local inner per core
SCALE = 8.0 / Dh
EPS = 1e-5
N_CORES = 8
TC = N // 128            # 16 token chunks
TG = N // 512            # 4 token groups
QC = N // 512            # 4 query chunks


def _install_ntff_hook_shim():
    """The agent image's antenv lacks axon_hooks; recreate it so
    run_bass_kernel_spmd(trace=True) can profile via libaxon_pjrt."""
    try:
        if "antenv.axon_hooks" in sys.modules:
            return True
        import antenv

        mod = types.ModuleType("antenv.axon_hooks")
        _state = {"hook": None}
        mod.set_axon_ntff_profile_hook = lambda h: _state.__setitem__("hook", h)
        mod.get_axon_ntff_profile_hook = lambda: _state["hook"]
        sys.modules["antenv.axon_hooks"] = mod
        antenv.axon_hooks = mod

        from trn_agent_boot.trn_boot import _ntff_profile_via_ctypes

        so = "/opt/axon/libaxon_pjrt.so"
        if os.path.exists(so):
            mod.set_axon_ntff_profile_hook(_ntff_profile_via_ctypes(so))
        return True
    except Exception:
        return False


_BUILD_CACHE = {}


def _build_program(flags):
    """Build + compile the per-core Bass program. flags: (use_bias_q/k/v,
    use_qnw, use_qnb, use_knw, use_knb) booleans for the general paths."""
    if flags in _BUILD_CACHE:
        return _BUILD_CACHE[flags]

    import concourse.bass as bass
    import concourse.bacc as bacc
    import concourse.mybir as mybir
    import concourse.tile as tile
    from concourse.masks import make_identity

    use_bq, use_bk, use_bv, use_qnw, use_qnb, use_knw, use_knb = flags
    f32 = mybir.dt.float32
    f32r = mybir.dt.float32r
    bf16 = mybir.dt.bfloat16

    nc = bacc.Bacc("TRN2", target_bir_lowering=False, debug=False,
                   num_devices=N_CORES)

    x_d = nc.dram_tensor("x", [N, D], f32, kind="ExternalInput")
    wq_d = nc.dram_tensor("wq", [D, IL], f32r, kind="ExternalInput")
    wk_d = nc.dram_tensor("wk", [D, IL], f32r, kind="ExternalInput")
    wv_d = nc.dram_tensor("wv", [D, IL], f32r, kind="ExternalInput")
    wo_d = nc.dram_tensor("wo", [IL, D], bf16, kind="ExternalInput")
    out_d = nc.dram_tensor("out", [N, D], f32, kind="ExternalOutput")
    extra_d = {}
    for name, used in (("bq", use_bq), ("bk", use_bk), ("bv", use_bv),
                       ("qnw", use_qnw), ("qnb", use_qnb),
                       ("knw", use_knw), ("knb", use_knb)):
        if used:
            extra_d[name] = nc.dram_tensor(name, [1, IL], f32, kind="ExternalInput")

    with tile.TileContext(nc) as tc:
        with tc.tile_pool(name="const", bufs=1) as constp, \
             tc.tile_pool(name="w", bufs=3) as wpool, \
             tc.tile_pool(name="x", bufs=1) as xpool, \
             tc.tile_pool(name="xt", bufs=1) as xtpool, \
             tc.tile_pool(name="qkt", bufs=1) as qktp, \
             tc.tile_pool(name="v", bufs=1) as vpool, \
             tc.tile_pool(name="ot", bufs=1) as otpool, \
             tc.tile_pool(name="pt", bufs=4) as ptpool, \
             tc.tile_pool(name="small", bufs=4) as small, \
             tc.tile_pool(name="outp", bufs=1) as outpool, \
             tc.tile_pool(name="ps", bufs=1, space="PSUM") as psp:

            # ---- constants ----
            ident_f = small.tile([128, 128], f32, tag="sq", bufs=1)
            make_identity(nc, ident_f)
            ident = constp.tile([128, 128], f32r, tag="ident")
            nc.vector.tensor_copy(ident, ident_f)
            maskp = constp.tile([128, 896], bf16, tag="mask")
            nc.gpsimd.memset(maskp, 1.0)
            # keep where (free idx u) - 384 - ki >= 0, else 0
            nc.gpsimd.affine_select(
                out=maskp, in_=maskp, compare_op=mybir.AluOpType.is_ge,
                fill=0.0, base=-384, channel_multiplier=-1, pattern=[[1, 896]])
            eps_t = constp.tile([128, 1], f32, tag="eps")
            nc.vector.memset(eps_t, EPS)
            onesc = None
            if use_bq or use_bk or use_bv:
                onesc_f = small.tile([1, 128], f32, tag="sq", bufs=1)
                nc.vector.memset(onesc_f, 1.0)
                onesc = constp.tile([1, 128], f32r, tag="onesc")
                nc.vector.tensor_copy(onesc, onesc_f)

            extra_sb = {}
            for name in ("bq", "bk", "bv"):
                if name in extra_d:
                    t = constp.tile([1, IL], f32r, tag=name)
                    tf = constp.tile([1, IL], f32, tag=name + "f")
                    nc.sync.dma_start(out=tf, in_=extra_d[name].ap())
                    nc.vector.tensor_copy(t, tf)  # round to f32r
                    extra_sb[name] = t
            for name in ("qnw", "qnb", "knw", "knb"):
                if name in extra_d:
                    row = constp.tile([1, IL], f32, tag=name + "r")
                    nc.sync.dma_start(out=row, in_=extra_d[name].ap())
                    t = constp.tile([128, IL], f32, tag=name)
                    nc.gpsimd.partition_broadcast(t, row)
                    extra_sb[name] = t

            # ---- weights ----
            wq_sb = wpool.tile([128, 8, IL], f32r, tag="w")
            wk_sb = wpool.tile([128, 8, IL], f32r, tag="w")
            wv_sb = wpool.tile([128, 8, IL], f32r, tag="w")
            for w_sb, w_d in ((wq_sb, wq_d), (wk_sb, wk_d), (wv_sb, wv_d)):
                nc.sync.dma_start(
                    out=w_sb, in_=w_d.ap().rearrange("(dc p) i -> p dc i", p=128))

            qt_sb = qktp.tile([128, 4, N], f32r, tag="qt")
            kt_sb = qktp.tile([128, 4, N], f32r, tag="kt")
            vaug = vpool.tile([128, TC, HL, 65], bf16, tag="vaug")
            nc.gpsimd.memset(vaug[:, :, :, 64:65], 1.0)
            ot_sb = otpool.tile([128, 4, N], bf16, tag="ot")

            # =================== Phase A: LN + transpose + QKV ===============
            for tg in range(TG):
                xt_g = xtpool.tile([128, 8, 512], f32r, tag="xt")
                for t in range(4):
                    tci = tg * 4 + t
                    x_t = xpool.tile([128, D], f32, tag="x")
                    nc.sync.dma_start(
                        out=x_t, in_=x_d.ap()[tci * 128:(tci + 1) * 128, :])
                    bn = small.tile([128, 2, 6], f32, tag="bn")
                    nc.vector.bn_stats(bn[:, 0, :], x_t[:, 0:512])
                    nc.vector.bn_stats(bn[:, 1, :], x_t[:, 512:1024])
                    mv = small.tile([128, 2], f32, tag="mv")
                    nc.vector.bn_aggr(mv, bn)
                    nc.scalar.activation(
                        out=mv[:, 1:2], in_=mv[:, 1:2],
                        func=mybir.ActivationFunctionType.Sqrt,
                        bias=eps_t, scale=1.0)
                    nc.vector.reciprocal(mv[:, 1:2], mv[:, 1:2])
                    x_r = small.tile([128, D], f32r, tag="xln", bufs=2)
                    nc.vector.tensor_scalar(
                        out=x_r, in0=x_t, scalar1=mv[:, 0:1], scalar2=mv[:, 1:2],
                        op0=mybir.AluOpType.subtract, op1=mybir.AluOpType.mult)
                    for half in range(2):
                        ps_tr = psp.tile([128, 4, 128], f32, tag="o", bufs=4)
                        for i in range(4):
                            dc = 4 * half + i
                            nc.tensor.transpose(
                                ps_tr.bitcast(f32r)[:, i, :],
                                x_r[:, dc * 128:(dc + 1) * 128], ident)
                        dst = xt_g[:, 4 * half:4 * half + 4,
                                   t * 128:(t + 1) * 128]
                        if half == 0:
                            nc.vector.tensor_copy(dst, ps_tr)
                        else:
                            nc.scalar.copy(dst, ps_tr)

                # QKV for this token group
                for t in range(4):
                    tci = tg * 4 + t
                    for proj, w_sb in (("q", wq_sb), ("k", wk_sb), ("v", wv_sb)):
                        ps = psp.tile([128, 512], f32, tag="o", bufs=4)
                        bias_sb = extra_sb.get("b" + proj)
                        for dc in range(8):
                            nc.tensor.matmul(
                                ps, lhsT=xt_g[:, dc, t * 128:(t + 1) * 128],
                                rhs=w_sb[:, dc, :],
                                start=(dc == 0),
                                stop=(dc == 7 and bias_sb is None))
                        if bias_sb is not None:
                            nc.tensor.matmul(ps, lhsT=onesc, rhs=bias_sb,
                                             start=False, stop=True)
                        if proj == "v":
                            nc.vector.tensor_copy(
                                out=vaug[:, tci, :, 0:64],
                                in_=ps.rearrange("p (h d) -> p h d", h=HL))
                            continue
                        # qk-LN: rstd from sumsq (mean removed via weights)
                        qraw = small.tile([128, 512], f32, tag="qraw", bufs=2)
                        nc.vector.tensor_copy(qraw, ps)
                        sq = small.tile([128, 512], f32, tag="sq", bufs=1)
                        nc.gpsimd.tensor_mul(sq, qraw, qraw)
                        ss = small.tile([128, HL], f32, tag="ss")
                        nc.vector.reduce_sum(
                            ss, sq.rearrange("p (h d) -> p h d", h=HL),
                            axis=mybir.AxisListType.X)
                        nc.scalar.activation(
                            out=ss, in_=ss,
                            func=mybir.ActivationFunctionType.Sqrt,
                            bias=eps_t, scale=1.0 / Dh)
                        nc.vector.reciprocal(ss, ss)
                        qln = small.tile([128, 512], f32r, tag="qln", bufs=2)
                        w_bc = extra_sb.get(proj + "nw")
                        b_bc = extra_sb.get(proj + "nb")
                        nc.vector.scalar_tensor_tensor(
                            out=qln.rearrange("p (h d) -> p h d", h=HL),
                            in0=qraw.rearrange("p (h d) -> p h d", h=HL),
                            scalar=1.0,
                            in1=ss.broadcast_to([128, HL, Dh]),
                            op0=mybir.AluOpType.mult, op1=mybir.AluOpType.mult)
                        if w_bc is not None:
                            nc.vector.tensor_mul(
                                qln, qln.bitcast(f32), w_bc)
                        if b_bc is not None:
                            nc.vector.tensor_add(
                                qln, qln.bitcast(f32), b_bc)
                        ps_tr = psp.tile([128, 4, 128], f32, tag="o", bufs=4)
                        for m in range(4):
                            nc.tensor.transpose(
                                ps_tr.bitcast(f32r)[:, m, :],
                                qln[:, m * 128:(m + 1) * 128], ident)
                        dst_t = qt_sb if proj == "q" else kt_sb
                        dst = dst_t[:, :, tci * 128:(tci + 1) * 128]
                        if proj == "q":
                            nc.vector.tensor_copy(dst, ps_tr)
                        else:
                            nc.scalar.copy(dst, ps_tr)

            wo_sb = wpool.tile([128, 4, D], bf16, tag="w")
            nc.sync.dma_start(
                out=wo_sb, in_=wo_d.ap().rearrange("(m p) i -> p m i", p=128))

            # =================== Phase B: attention ==========================
            # kc pairs share one 2-bank PSUM tile so exp amortizes the 352-cyc
            # ACTIVATE overhead; the diagonal pair is q-sliced to skip fully
            # masked columns. O^T and denominators are staged raw; the
            # reciprocal runs once per qc on all 8 head rows, normalization is
            # a deferred in-place pass over ot_sb.
            for qc in range(QC):
                nkc = 4 * (qc + 1)
                for hp in range(4):
                    ps_o = [psp.tile([65, 512], f32, tag="o", bufs=4,
                                     name=f"ps_o_{qc}_{hp}_{s}")
                            for s in range(2)]
                    for kcg in range(nkc // 2):
                        kc0 = 2 * kcg
                        diag2 = (kc0 * 128 - qc * 512) == 256  # deltas 256,384
                        q0 = 256 if diag2 else 0  # valid q-cols [q0:512]
                        qw = 512 - q0
                        pts = []
                        sss = []
                        for sub in range(2):
                            r0 = 64 * sub
                            ps_s = psp.tile([128, 2, 512], f32, tag="s",
                                            bufs=2,
                                            name=f"ps_s_{qc}_{hp}_{kcg}_{sub}")
                            for i in range(2):
                                kc = kc0 + i
                                nc.tensor.matmul(
                                    ps_s[:, i, q0:512],
                                    lhsT=kt_sb[r0:r0 + 64, hp,
                                               kc * 128:(kc + 1) * 128],
                                    rhs=qt_sb[r0:r0 + 64, hp,
                                              qc * 512 + q0:(qc + 1) * 512],
                                    start=True, stop=True)
                            sss.append(ps_s)
                        for sub in range(2):
                            ps_s = sss[sub]
                            pt = ptpool.tile([128, 2, 512], bf16, tag="pt",
                                             bufs=3,
                                             name=f"pt_{qc}_{hp}_{kcg}_{sub}")
                            nc.scalar.activation(
                                out=pt[:, :, q0:512], in_=ps_s[:, :, q0:512],
                                func=mybir.ActivationFunctionType.Exp,
                                scale=SCALE)
                            delta0 = kc0 * 128 - qc * 512
                            if delta0 >= 0:
                                # masks for the two sub-tiles are shifted
                                # slices of maskp: offset 384-delta0+q0, the
                                # second sub-tile 128 further left
                                moff = 384 - delta0 + q0
                                mask_ap = bass.AP(
                                    tensor=maskp.tensor,
                                    offset=maskp.offset + moff,
                                    ap=[maskp.ap[0], [-128, 2], [1, qw]])
                                nc.vector.tensor_mul(
                                    pt[:, :, q0:512], pt[:, :, q0:512],
                                    mask_ap)
                            pts.append(pt)
                        for sub in range(2):
                            h = 2 * hp + sub
                            for i in range(2):
                                kc = kc0 + i
                                nc.tensor.matmul(
                                    ps_o[sub][:, q0:512],
                                    lhsT=vaug[:, kc, h, :],
                                    rhs=pts[sub][:, i, q0:512],
                                    start=(kc == 0), stop=(kc == nkc - 1))
                    for sub in range(2):
                        r0 = 64 * sub
                        r = small.tile([1, 512], f32, tag="r", bufs=2,
                                       name=f"r_{qc}_{hp}_{sub}")
                        nc.vector.tensor_copy(r, ps_o[sub][64:65, :])
                        nc.vector.reciprocal_approx_fast(r, r)
                        rb = small.tile([128, 512], f32, tag="rb", bufs=2,
                                        name=f"rb_{qc}_{hp}_{sub}")
                        nc.gpsimd.partition_broadcast(rb, r)
                        nc.vector.scalar_tensor_tensor(
                            out=ot_sb[r0:r0 + 64, hp,
                                      qc * 512:(qc + 1) * 512],
                            in0=ps_o[sub][0:64, :], scalar=1.0,
                            in1=rb[0:64, :],
                            op0=mybir.AluOpType.mult,
                            op1=mybir.AluOpType.mult)

            # =================== Phase C: out-proj ===========================
            for tci in range(TC):
                out_sb = outpool.tile([128, D], f32, tag="osb")
                for dch in range(2):
                    ps = psp.tile([128, 512], f32, tag="o", bufs=4)
                    for m in range(4):
                        nc.tensor.matmul(
                            ps, lhsT=ot_sb[:, m, tci * 128:(tci + 1) * 128],
                            rhs=wo_sb[:, m, dch * 512:(dch + 1) * 512],
                            start=(m == 0), stop=(m == 3))
                    if dch == 0:
                        nc.vector.tensor_copy(
                            out_sb[:, dch * 512:(dch + 1) * 512], ps)
                    else:
                        nc.scalar.copy(
                            out_sb[:, dch * 512:(dch + 1) * 512], ps)
                nc.sync.dma_start(
                    out=out_d.ap()[tci * 128:(tci + 1) * 128, :], in_=out_sb)

    nc.compile()
    _BUILD_CACHE[flags] = nc
    return nc


def kernel(**inputs):
    x = np.ascontiguousarray(np.asarray(inputs["x"], np.float32))
    ln_w = np.asarray(inputs["ln_w"], np.float32)
    ln_b = np.asarray(inputs["ln_b"], np.float32)
    Wq = np.asarray(inputs["Wq"], np.float32)
    Wk = np.asarray(inputs["Wk"], np.float32)
    Wv = np.asarray(inputs["Wv"], np.float32)
    qn_w = np.asarray(inputs["qn_w"], np.float32)
    qn_b = np.asarray(inputs["qn_b"], np.float32)
    kn_w = np.asarray(inputs["kn_w"], np.float32)
    kn_b = np.asarray(inputs["kn_b"], np.float32)
    Wo = np.asarray(inputs["Wo"], np.float32)
    bo = np.asarray(inputs["bo"], np.float32)

    # ---- host-side weight folding ----
    def fold(W):
        return ln_w[:, None] * W, ln_b @ W

    W1q, bq = fold(Wq)
    W1k, bk = fold(Wk)
    W1v, bv = fold(Wv)

    def center(W, b):
        W3 = W.reshape(D, H, Dh)
        W3 = W3 - W3.mean(-1, keepdims=True)
        b3 = b.reshape(H, Dh)
        b3 = b3 - b3.mean(-1, keepdims=True)
        return np.ascontiguousarray(W3.reshape(D, H * Dh)), b3.reshape(H * Dh)

    W1q, bq = center(W1q, bq)
    W1k, bk = center(W1k, bk)

    flags = (
        bool(np.any(bq)), bool(np.any(bk)), bool(np.any(bv)),
        not np.all(qn_w == 1.0), bool(np.any(qn_b)),
        not np.all(kn_w == 1.0), bool(np.any(kn_b)),
    )
    nc = _build_program(flags)

    wo_bf = Wo.astype(ml_dtypes.bfloat16)
    in_maps = []
    for c in range(N_CORES):
        b, g = c // 2, c % 2
        sl = slice(IL * g, IL * (g + 1))
        m = {
            "x": x[b],
            "wq": np.ascontiguousarray(W1q[:, sl]),
            "wk": np.ascontiguousarray(W1k[:, sl]),
            "wv": np.ascontiguousarray(W1v[:, sl]),
            "wo": np.ascontiguousarray(wo_bf[sl, :]),
        }
        if flags[0]:
            m["bq"] = np.ascontiguousarray(bq[None, sl])
        if flags[1]:
            m["bk"] = np.ascontiguousarray(bk[None, sl])
        if flags[2]:
            m["bv"] = np.ascontiguousarray(bv[None, sl])
        if flags[3]:
            m["qnw"] = np.ascontiguousarray(np.tile(qn_w, HL)[None, :])
        if flags[4]:
            m["qnb"] = np.ascontiguousarray(np.tile(qn_b, HL)[None, :])
        if flags[5]:
            m["knw"] = np.ascontiguousarray(np.tile(kn_w, HL)[None, :])
        if flags[6]:
            m["knb"] = np.ascontiguousarray(np.tile(kn_b, HL)[None, :])
        in_maps.append(m)

    from concourse.bass_utils import run_bass_kernel_spmd

    trace = _install_ntff_hook_shim() and \
        os.environ.get("KERNEL_NO_TRACE", "0") != "1"
    try:
        res = run_bass_kernel_spmd(
            nc, in_maps, core_ids=list(range(N_CORES)), trace=trace)
    except Exception:
        if not trace:
            raise
        res = run_bass_kernel_spmd(
            nc, in_maps, core_ids=list(range(N_CORES)), trace=False)
    globals()["LAST_RESULT"] = res
    if res.exec_time_ns is not None:
        print(f"HW exec time: {res.exec_time_ns} ns")

    out = np.zeros((B, N, D), np.float32)
    for b in range(B):
        out[b] = res.results[2 * b]["out"] + res.results[2 * b + 1]["out"]
    out += bo
    return out


# revision 19
# speedup vs baseline: 1.5517x; 1.5517x over previous
"""Trainium2 Bass kernel for nn_Attention (B=4, N=2048, D=1024, H=16, Dh=64).

Distribution over 8 NeuronCores: 4-way data parallel on batch x 2-way tensor
parallel on heads (8 heads / 512 inner dims per core). Each core computes a
partial output projection; the host sums the two head-group partials per batch
and adds bo.

Per-core dataflow (all matmuls f32r or bf16 at 1 cycle/row on the PE):
  A) x-LN (mean/var only; ln_w/ln_b folded into the projection weights on the
     host) -> PE-transpose x_ln -> XT [D, tokens]
  A2) Q/K/V token-major via XT-stationary matmuls. qk-LN mean is removed by
     host-side per-head column centering of Wq/Wk, so only sumsq/rstd is
     computed on device. Q/K are then PE-transposed into QT/KT [inner, tokens].
  B) attention in S^T layout: S^T[keys,q] = K @ Q^T, exp on ACT (no max
     subtraction -- |score| <= 8 by LN bounds), causal mask as 0/1 multiply on
     diagonal tiles, P^T@V with a fused ones-column for the softmax denominator,
     normalization via gpsimd partition-broadcast of 1/denom.
  C) out-proj: out = (O^T)^T @ Wo in bf16.
"""

import os
import sys
import types

import numpy as np
import ml_dtypes

B, N, D = 4, 2048, 1024
H, Dh = 16, 64
HL, IL = 8, 512          # local heads / local inner per core
SCALE = 8.0 / Dh
EPS = 1e-5
N_CORES = 8
TC = N // 128            # 16 token chunks
TG = N // 512            # 4 token groups
QC = N // 512            # 4 query chunks


def _install_ntff_hook_shim():
    """The agent image's antenv lacks axon_hooks; recreate it so
    run_bass_kernel_spmd(trace=True) can profile via libaxon_pjrt."""
    try:
        if "antenv.axon_hooks" in sys.modules:
            return True
        import antenv

        mod = types.ModuleType("antenv.axon_hooks")
        _state = {"hook": None}
        mod.set_axon_ntff_profile_hook = lambda h: _state.__setitem__("hook", h)
        mod.get_axon_ntff_profile_hook = lambda: _state["hook"]
        sys.modules["antenv.axon_hooks"] = mod
        antenv.axon_hooks = mod

        from trn_agent_boot.trn_boot import _ntff_profile_via_ctypes

        so = "/opt/axon/libaxon_pjrt.so"
        if os.path.exists(so):
            mod.set_axon_ntff_profile_hook(_ntff_profile_via_ctypes(so))
        return True
    except Exception:
        return False


_BUILD_CACHE = {}


def _build_program(flags):
    """Build + compile the per-core Bass program. flags: (use_bias_q/k/v,
    use_qnw, use_qnb, use_knw, use_knb) booleans for the general paths."""
    if flags in _BUILD_CACHE:
        return _BUILD_CACHE[flags]

    import concourse.bass as bass
    import concourse.bacc as bacc
    import concourse.mybir as mybir
    import concourse.tile as tile
    from concourse.masks import make_identity

    use_bq, use_bk, use_bv, use_qnw, use_qnb, use_knw, use_knb = flags
    f32 = mybir.dt.float32
    f32r = mybir.dt.float32r
    bf16 = mybir.dt.bfloat16

    nc = bacc.Bacc("TRN2", target_bir_lowering=False, debug=False,
                   num_devices=N_CORES)

    x_d = nc.dram_tensor("x", [N, D], f32, kind="ExternalInput")
    wq_d = nc.dram_tensor("wq", [D, IL], f32r, kind="ExternalInput")
    wk_d = nc.dram_tensor("wk", [D, IL], f32r, kind="ExternalInput")
    wv_d = nc.dram_tensor("wv", [D, IL], f32r, kind="ExternalInput")
    wo_d = nc.dram_tensor("wo", [IL, D], bf16, kind="ExternalInput")
    out_d = nc.dram_tensor("out", [N, D], f32, kind="ExternalOutput")
    extra_d = {}
    for name, used in (("bq", use_bq), ("bk", use_bk), ("bv", use_bv),
                       ("qnw", use_qnw), ("qnb", use_qnb),
                       ("knw", use_knw), ("knb", use_knb)):
        if used:
            extra_d[name] = nc.dram_tensor(name, [1, IL], f32, kind="ExternalInput")

    with tile.TileContext(nc) as tc:
        with tc.tile_pool(name="const", bufs=1) as constp, \
             tc.tile_pool(name="w", bufs=3) as wpool, \
             tc.tile_pool(name="x", bufs=1) as xpool, \
             tc.tile_pool(name="xt", bufs=1) as xtpool, \
             tc.tile_pool(name="qkt", bufs=1) as qktp, \
             tc.tile_pool(name="v", bufs=1) as vpool, \
             tc.tile_pool(name="ot", bufs=1) as otpool, \
             tc.tile_pool(name="pt", bufs=4) as ptpool, \
             tc.tile_pool(name="small", bufs=4) as small, \
             tc.tile_pool(name="outp", bufs=1) as outpool, \
             tc.tile_pool(name="ps", bufs=1, space="PSUM") as psp:

            # ---- constants ----
            ident_f = small.tile([128, 128], f32, tag="sq", bufs=1)
            make_identity(nc, ident_f)
            ident = constp.tile([128, 128], f32r, tag="ident")
            nc.vector.tensor_copy(ident, ident_f)
            maskp = constp.tile([128, 896], bf16, tag="mask")
            nc.gpsimd.memset(maskp, 1.0)
            # keep where (free idx u) - 384 - ki >= 0, else 0
            nc.gpsimd.affine_select(
                out=maskp, in_=maskp, compare_op=mybir.AluOpType.is_ge,
                fill=0.0, base=-384, channel_multiplier=-1, pattern=[[1, 896]])
            eps_t = constp.tile([128, 1], f32, tag="eps")
            nc.vector.memset(eps_t, EPS)
            onesc = None
            if use_bq or use_bk or use_bv:
                onesc_f = small.tile([1, 128], f32, tag="sq", bufs=1)
                nc.vector.memset(onesc_f, 1.0)
                onesc = constp.tile([1, 128], f32r, tag="onesc")
                nc.vector.tensor_copy(onesc, onesc_f)

            extra_sb = {}
            for name in ("bq", "bk", "bv"):
                if name in extra_d:
                    t = constp.tile([1, IL], f32r, tag=name)
                    tf = constp.tile([1, IL], f32, tag=name + "f")
                    nc.sync.dma_start(out=tf, in_=extra_d[name].ap())
                    nc.vector.tensor_copy(t, tf)  # round to f32r
                    extra_sb[name] = t
            for name in ("qnw", "qnb", "knw", "knb"):
                if name in extra_d:
                    row = constp.tile([1, IL], f32, tag=name + "r")
                    nc.sync.dma_start(out=row, in_=extra_d[name].ap())
                    t = constp.tile([128, IL], f32, tag=name)
                    nc.gpsimd.partition_broadcast(t, row)
                    extra_sb[name] = t

            # ---- weights ----
            wq_sb = wpool.tile([128, 8, IL], f32r, tag="w")
            wk_sb = wpool.tile([128, 8, IL], f32r, tag="w")
            wv_sb = wpool.tile([128, 8, IL], f32r, tag="w")
            for w_sb, w_d in ((wq_sb, wq_d), (wk_sb, wk_d), (wv_sb, wv_d)):
                nc.sync.dma_start(
                    out=w_sb, in_=w_d.ap().rearrange("(dc p) i -> p dc i", p=128))

            qt_sb = qktp.tile([128, 4, N], f32r, tag="qt")
            kt_sb = qktp.tile([128, 4, N], f32r, tag="kt")
            vaug = vpool.tile([128, TC, HL, 65], bf16, tag="vaug")
            nc.gpsimd.memset(vaug[:, :, :, 64:65], 1.0)
            ot_sb = otpool.tile([128, 4, N], bf16, tag="ot")

            # =================== Phase A: LN + transpose + QKV ===============
            for tg in range(TG):
                xt_g = xtpool.tile([128, 8, 512], f32r, tag="xt")
                for t in range(4):
                    tci = tg * 4 + t
                    x_t = xpool.tile([128, D], f32, tag="x")
                    nc.sync.dma_start(
                        out=x_t, in_=x_d.ap()[tci * 128:(tci + 1) * 128, :])
                    bn = small.tile([128, 2, 6], f32, tag="bn")
                    nc.vector.bn_stats(bn[:, 0, :], x_t[:, 0:512])
                    nc.vector.bn_stats(bn[:, 1, :], x_t[:, 512:1024])
                    mv = small.tile([128, 2], f32, tag="mv")
                    nc.vector.bn_aggr(mv, bn)
                    nc.scalar.activation(
                        out=mv[:, 1:2], in_=mv[:, 1:2],
                        func=mybir.ActivationFunctionType.Sqrt,
                        bias=eps_t, scale=1.0)
                    nc.vector.reciprocal(mv[:, 1:2], mv[:, 1:2])
                    x_r = small.tile([128, D], f32r, tag="xln", bufs=2)
                    nc.vector.tensor_scalar(
                        out=x_r, in0=x_t, scalar1=mv[:, 0:1], scalar2=mv[:, 1:2],
                        op0=mybir.AluOpType.subtract, op1=mybir.AluOpType.mult)
                    for half in range(2):
                        ps_tr = psp.tile([128, 4, 128], f32, tag="o", bufs=2)
                        for i in range(4):
                            dc = 4 * half + i
                            nc.tensor.transpose(
                                ps_tr.bitcast(f32r)[:, i, :],
                                x_r[:, dc * 128:(dc + 1) * 128], ident)
                        dst = xt_g[:, 4 * half:4 * half + 4,
                                   t * 128:(t + 1) * 128]
                        if half == 0:
                            nc.vector.tensor_copy(dst, ps_tr)
                        else:
                            nc.scalar.copy(dst, ps_tr)

                # QKV for this token group
                for t in range(4):
                    tci = tg * 4 + t
                    for proj, w_sb in (("q", wq_sb), ("k", wk_sb), ("v", wv_sb)):
                        ps = psp.tile([128, 512], f32, tag="acc", bufs=2)
                        bias_sb = extra_sb.get("b" + proj)
                        for dc in range(8):
                            nc.tensor.matmul(
                                ps, lhsT=xt_g[:, dc, t * 128:(t + 1) * 128],
                                rhs=w_sb[:, dc, :],
                                start=(dc == 0),
                                stop=(dc == 7 and bias_sb is None))
                        if bias_sb is not None:
                            nc.tensor.matmul(ps, lhsT=onesc, rhs=bias_sb,
                                             start=False, stop=True)
                        if proj == "v":
                            nc.vector.tensor_copy(
                                out=vaug[:, tci, :, 0:64],
                                in_=ps.rearrange("p (h d) -> p h d", h=HL))
                            continue
                        # qk-LN: rstd from sumsq (mean removed via weights)
                        qraw = small.tile([128, 512], f32, tag="qraw", bufs=2)
                        nc.vector.tensor_copy(qraw, ps)
                        sq = small.tile([128, 512], f32, tag="sq", bufs=1)
                        nc.gpsimd.tensor_mul(sq, qraw, qraw)
                        ss = small.tile([128, HL], f32, tag="ss")
                        nc.vector.reduce_sum(
                            ss, sq.rearrange("p (h d) -> p h d", h=HL),
                            axis=mybir.AxisListType.X)
                        nc.scalar.activation(
                            out=ss, in_=ss,
                            func=mybir.ActivationFunctionType.Sqrt,
                            bias=eps_t, scale=1.0 / Dh)
                        nc.vector.reciprocal(ss, ss)
                        qln = small.tile([128, 512], f32r, tag="qln", bufs=2)
                        w_bc = extra_sb.get(proj + "nw")
                        b_bc = extra_sb.get(proj + "nb")
                        nc.vector.scalar_tensor_tensor(
                            out=qln.rearrange("p (h d) -> p h d", h=HL),
                            in0=qraw.rearrange("p (h d) -> p h d", h=HL),
                            scalar=1.0,
                            in1=ss.broadcast_to([128, HL, Dh]),
                            op0=mybir.AluOpType.mult, op1=mybir.AluOpType.mult)
                        if w_bc is not None:
                            nc.vector.tensor_mul(
                                qln, qln.bitcast(f32), w_bc)
                        if b_bc is not None:
                            nc.vector.tensor_add(
                                qln, qln.bitcast(f32), b_bc)
                        ps_tr = psp.tile([128, 4, 128], f32, tag="o", bufs=2)
                        for m in range(4):
                            nc.tensor.transpose(
                                ps_tr.bitcast(f32r)[:, m, :],
                                qln[:, m * 128:(m + 1) * 128], ident)
                        dst_t = qt_sb if proj == "q" else kt_sb
                        dst = dst_t[:, :, tci * 128:(tci + 1) * 128]
                        if proj == "q":
                            nc.vector.tensor_copy(dst, ps_tr)
                        else:
                            nc.scalar.copy(dst, ps_tr)

            wo_sb = wpool.tile([128, 4, D], bf16, tag="w")
            nc.sync.dma_start(
                out=wo_sb, in_=wo_d.ap().rearrange("(m p) i -> p m i", p=128))

            # =================== Phase B: attention ==========================
            # kc pairs share one 2-bank PSUM tile so exp amortizes the 352-cyc
            # ACTIVATE overhead; the diagonal pair is q-sliced to skip fully
            # masked columns. O^T and denominators are staged raw; the
            # reciprocal runs once per qc on all 8 head rows, normalization is
            # a deferred in-place pass over ot_sb.
            for qc in range(QC):
                nkc = 4 * (qc + 1)
                for hp in range(4):
                    ps_o = [psp.tile([65, 512], f32, tag="o", bufs=2,
                                     name=f"ps_o_{qc}_{hp}_{s}")
                            for s in range(2)]
                    for kcg in range(nkc // 2):
                        kc0 = 2 * kcg
                        diag2 = (kc0 * 128 - qc * 512) == 256  # deltas 256,384
                        q0 = 256 if diag2 else 0  # valid q-cols [q0:512]
                        qw = 512 - q0
                        pts = []
                        sss = []
                        for sub in range(2):
                            r0 = 64 * sub
                            ps_s = psp.tile([128, 2, 512], f32, tag="s",
                                            bufs=2,
                                            name=f"ps_s_{qc}_{hp}_{kcg}_{sub}")
                            for i in range(2):
                                kc = kc0 + i
                                nc.tensor.matmul(
                                    ps_s[:, i, q0:512],
                                    lhsT=kt_sb[r0:r0 + 64, hp,
                                               kc * 128:(kc + 1) * 128],
                                    rhs=qt_sb[r0:r0 + 64, hp,
                                              qc * 512 + q0:(qc + 1) * 512],
                                    start=True, stop=True)
                            sss.append(ps_s)
                        for sub in range(2):
                            ps_s = sss[sub]
                            pt = ptpool.tile([128, 2, 512], bf16, tag="pt",
                                             bufs=3,
                                             name=f"pt_{qc}_{hp}_{kcg}_{sub}")
                            nc.scalar.activation(
                                out=pt[:, :, q0:512], in_=ps_s[:, :, q0:512],
                                func=mybir.ActivationFunctionType.Exp,
                                scale=SCALE)
                            delta0 = kc0 * 128 - qc * 512
                            if delta0 >= 0:
                                # masks for the two sub-tiles are shifted
                                # slices of maskp: offset 384-delta0+q0, the
                                # second sub-tile 128 further left
                                moff = 384 - delta0 + q0
                                mask_ap = bass.AP(
                                    tensor=maskp.tensor,
                                    offset=maskp.offset + moff,
                                    ap=[maskp.ap[0], [-128, 2], [1, qw]])
                                nc.vector.tensor_mul(
                                    pt[:, :, q0:512], pt[:, :, q0:512],
                                    mask_ap)
                            pts.append(pt)
                        for sub in range(2):
                            h = 2 * hp + sub
                            for i in range(2):
                                kc = kc0 + i
                                nc.tensor.matmul(
                                    ps_o[sub][:, q0:512],
                                    lhsT=vaug[:, kc, h, :],
                                    rhs=pts[sub][:, i, q0:512],
                                    start=(kc == 0), stop=(kc == nkc - 1))
                    for sub in range(2):
                        r0 = 64 * sub
                        r = small.tile([1, 512], f32, tag="r", bufs=2,
                                       name=f"r_{qc}_{hp}_{sub}")
                        nc.vector.tensor_copy(r, ps_o[sub][64:65, :])
                        nc.vector.reciprocal_approx_fast(r, r)
                        rb = small.tile([128, 512], f32, tag="rb", bufs=2,
                                        name=f"rb_{qc}_{hp}_{sub}")
                        nc.gpsimd.partition_broadcast(rb, r)
                        nc.vector.scalar_tensor_tensor(
                            out=ot_sb[r0:r0 + 64, hp,
                                      qc * 512:(qc + 1) * 512],
                            in0=ps_o[sub][0:64, :], scalar=1.0,
                            in1=rb[0:64, :],
                            op0=mybir.AluOpType.mult,
                            op1=mybir.AluOpType.mult)

            # =================== Phase C: out-proj ===========================
            for tci in range(TC):
                out_sb = outpool.tile([128, D], f32, tag="osb")
                for dch in range(2):
                    ps = psp.tile([128, 512], f32, tag="acc", bufs=2)
                    for m in range(4):
                        nc.tensor.matmul(
                            ps, lhsT=ot_sb[:, m, tci * 128:(tci + 1) * 128],
                            rhs=wo_sb[:, m, dch * 512:(dch + 1) * 512],
                            start=(m == 0), stop=(m == 3))
                    if dch == 0:
                        nc.vector.tensor_copy(
                            out_sb[:, dch * 512:(dch + 1) * 512], ps)
                    else:
                        nc.scalar.copy(
                            out_sb[:, dch * 512:(dch + 1) * 512], ps)
                nc.sync.dma_start(
                    out=out_d.ap()[tci * 128:(tci + 1) * 128, :], in_=out_sb)

    nc.compile()
    _BUILD_CACHE[flags] = nc
    return nc


def kernel(**inputs):
    x = np.ascontiguousarray(np.asarray(inputs["x"], np.float32))
    ln_w = np.asarray(inputs["ln_w"], np.float32)
    ln_b = np.asarray(inputs["ln_b"], np.float32)
    Wq = np.asarray(inputs["Wq"], np.float32)
    Wk = np.asarray(inputs["Wk"], np.float32)
    Wv = np.asarray(inputs["Wv"], np.float32)
    qn_w = np.asarray(inputs["qn_w"], np.float32)
    qn_b = np.asarray(inputs["qn_b"], np.float32)
    kn_w = np.asarray(inputs["kn_w"], np.float32)
    kn_b = np.asarray(inputs["kn_b"], np.float32)
    Wo = np.asarray(inputs["Wo"], np.float32)
    bo = np.asarray(inputs["bo"], np.float32)

    # ---- host-side weight folding ----
    def fold(W):
        return ln_w[:, None] * W, ln_b @ W

    W1q, bq = fold(Wq)
    W1k, bk = fold(Wk)
    W1v, bv = fold(Wv)

    def center(W, b):
        W3 = W.reshape(D, H, Dh)
        W3 = W3 - W3.mean(-1, keepdims=True)
        b3 = b.reshape(H, Dh)
        b3 = b3 - b3.mean(-1, keepdims=True)
        return np.ascontiguousarray(W3.reshape(D, H * Dh)), b3.reshape(H * Dh)

    W1q, bq = center(W1q, bq)
    W1k, bk = center(W1k, bk)

    flags = (
        bool(np.any(bq)), bool(np.any(bk)), bool(np.any(bv)),
        not np.all(qn_w == 1.0), bool(np.any(qn_b)),
        not np.all(kn_w == 1.0), bool(np.any(kn_b)),
    )
    nc = _build_program(flags)

    wo_bf = Wo.astype(ml_dtypes.bfloat16)
    in_maps = []
    for c in range(N_CORES):
        b, g = c // 2, c % 2
        sl = slice(IL * g, IL * (g + 1))
        m = {
            "x": x[b],
            "wq": np.ascontiguousarray(W1q[:, sl]),
            "wk": np.ascontiguousarray(W1k[:, sl]),
            "wv": np.ascontiguousarray(W1v[:, sl]),
            "wo": np.ascontiguousarray(wo_bf[sl, :]),
        }
        if flags[0]:
            m["bq"] = np.ascontiguousarray(bq[None, sl])
        if flags[1]:
            m["bk"] = np.ascontiguousarray(bk[None, sl])
        if flags[2]:
            m["bv"] = np.ascontiguousarray(bv[None, sl])
        if flags[3]:
            m["qnw"] = np.ascontiguousarray(np.tile(qn_w, HL)[None, :])
        if flags[4]:
            m["qnb"] = np.ascontiguousarray(np.tile(qn_b, HL)[None, :])
        if flags[5]:
            m["knw"] = np.ascontiguousarray(np.tile(kn_w, HL)[None, :])
        if flags[6]:
            m["knb"] = np.ascontiguousarray(np.tile(kn_b, HL)[None, :])
        in_maps.append(m)

    from concourse.bass_utils import run_bass_kernel_spmd

    trace = _install_ntff_hook_shim() and \
        os.environ.get("KERNEL_NO_TRACE", "0") != "1"
    try:
        res = run_bass_kernel_spmd(
            nc, in_maps, core_ids=list(range(N_CORES)), trace=trace)
    except Exception:
        if not trace:
            raise
        res = run_bass_kernel_spmd(
            nc, in_maps, core_ids=list(range(N_CORES)), trace=False)
    globals()["LAST_RESULT"] = res
    if res.exec_time_ns is not None:
        print(f"HW exec time: {res.exec_time_ns} ns")

    out = np.zeros((B, N, D), np.float32)
    for b in range(B):
        out[b] = res.results[2 * b]["out"] + res.results[2 * b + 1]["out"]
    out += bo
    return out
